# revision 2
# baseline (speedup 1.0000x reference)
"""Trainium2 Bass kernel for the FlowNet-style correlation layer.

Problem (hardcoded):
  x_1, x_2, p_1, p_2: [1, 64, 96, 96] f32;  img: [1, 1, 96, 96] f32
  x1 = concat(x_1, p_1) -> [1,128,96,96];  x2 = pad(concat(x_2,p_2), 20)
  out_vb[d, h, w]  = sum_c x1[c,h,w] * x2[c, h+dy, w+dx],  d = dy*41+dx
  out_img[d, h, w] = pad(img,20)[h+dy, w+dx]
  returns (out_vb [1,1681,96,96], out_img [1,1681,96,96])

Strategy: shard over output rows h (12 per core, 8 cores). Per (h, dy)
the correlation row-pair Gram matrix G[w, wp] = sum_c x1[c,h,w] *
x2[c, h+dy-20, wp] is computed on the TensorEngine (contraction over the
128-channel partition dim). The diagonal band out[dx, w] = G[w, w+dx-20]
couples (partition, free) axes and is not expressible as one affine DMA,
so the device writes the rectangular Gram superset [h, w, dy, wp] and the
host extracts the band with a strided view while unsharding (out-of-band
entries fall in zero margins). out_img is produced by one overlapping-read
DRAM->DRAM replication DMA; the host zeroes the w-wraparound entries.
"""

import numpy as np

import concourse.bass as bass
import concourse.tile as tile
from concourse import bacc, mybir
from concourse.bass_types import AP
from concourse.bass_utils import run_bass_kernel_spmd

F32 = mybir.dt.float32

H = W = 96
C2 = 128           # concat channels
PAD = 20
D = 2 * PAD + 1    # 41 displacements per axis
NCORES = 8
HS = H // NCORES   # 12 output rows per core
X2R = HS + 2 * PAD # 52 x2 rows needed per core
IMR = X2R + 2      # 54 img rows (one extra margin row top+bottom)
DYB = 5            # dy rows per matmul (N = 480 <= 512 fp32 limit)

# matmul precision knob: "fp32" (exact, 4 cyc/row), "fp32r" (~1 cyc/row,
# reduced mantissa), "bf16x3" (3-pass bf16 hi/lo split, ~fp32 accuracy)
MM_DTYPE = "bf16x3"


def _dy_batches():
    out = []
    dy0 = 0
    while dy0 < D:
        nb = min(DYB, D - dy0)
        out.append((dy0, nb))
        dy0 += nb
    return out


def _build_nc():
    nc = bacc.Bacc("TRN2", target_bir_lowering=False, debug=False,
                   num_devices=NCORES)

    if MM_DTYPE == "bf16x3":
        bf = mybir.dt.bfloat16
        x1h = nc.declare_dram_parameter("x1h", [C2, HS * W], bf, isOutput=False)
        x1l = nc.declare_dram_parameter("x1l", [C2, HS * W], bf, isOutput=False)
        x2h = nc.declare_dram_parameter("x2h", [C2, X2R * W], bf, isOutput=False)
        x2l = nc.declare_dram_parameter("x2l", [C2, X2R * W], bf, isOutput=False)
    else:
        x1 = nc.declare_dram_parameter("x1", [C2, HS * W], F32, isOutput=False)
        x2 = nc.declare_dram_parameter("x2", [C2, X2R * W], F32, isOutput=False)
    img = nc.declare_dram_parameter("img", [IMR * W], F32, isOutput=False)
    corr = nc.declare_dram_parameter("corr", [HS, W, D, W], F32, isOutput=True)
    imgo = nc.declare_dram_parameter("imgo", [D, D, HS, W], F32, isOutput=True)

    with tile.TileContext(nc) as tc:
        with (
            tc.tile_pool(name="inp", bufs=1) as pin,
            tc.tile_pool(name="stage", bufs=3) as pst,
            tc.tile_pool(name="psum", bufs=8, space="PSUM") as pps,
        ):
            if MM_DTYPE == "bf16x3":
                bf = mybir.dt.bfloat16
                x1h_sb = pin.tile([C2, HS * W], bf)
                nc.sync.dma_start(x1h_sb[:], x1h[:])
                x1l_sb = pin.tile([C2, HS * W], bf)
                nc.sync.dma_start(x1l_sb[:], x1l[:])
                x2h_sb = pin.tile([C2, X2R * W], bf)
                nc.sync.dma_start(x2h_sb[:], x2h[:])
                x2l_sb = pin.tile([C2, X2R * W], bf)
                nc.sync.dma_start(x2l_sb[:], x2l[:])
            else:
                x1_sb = pin.tile([C2, HS * W], F32)
                nc.sync.dma_start(x1_sb[:], x1[:])
                x2_sb = pin.tile([C2, X2R * W], F32)
                nc.sync.dma_start(x2_sb[:], x2[:])

            # out_img: every (dy,dx) window is a contiguous [HS*W] run of the
            # h-padded image slab starting at (dy+1)*W + dx - PAD.
            img_src = AP(tensor=img[:].tensor, offset=W - PAD,
                         ap=[[W, D], [1, D], [1, HS * W]])
            img_dst = AP(tensor=imgo[:].tensor, offset=0,
                         ap=[[D * HS * W, D], [HS * W, D], [1, HS * W]])
            nc.sync.dma_start(img_dst, img_src)

            batches = _dy_batches()
            for h in range(HS):
                stage = pst.tile([W, D * W], F32)
                for b, (dy0, nb) in enumerate(batches):
                    ps = pps.tile([W, DYB * W], F32)
                    pslice = ps[:, : nb * W]
                    r0 = (h + dy0) * W
                    r1 = (h + dy0 + nb) * W
                    if MM_DTYPE == "bf16x3":
                        nc.tensor.matmul(pslice, x1h_sb[:, h * W:(h + 1) * W],
                                         x2h_sb[:, r0:r1],
                                         start=True, stop=False)
                        nc.tensor.matmul(pslice, x1h_sb[:, h * W:(h + 1) * W],
                                         x2l_sb[:, r0:r1],
                                         start=False, stop=False)
                        nc.tensor.matmul(pslice, x1l_sb[:, h * W:(h + 1) * W],
                                         x2h_sb[:, r0:r1],
                                         start=False, stop=True)
                    elif MM_DTYPE == "fp32r":
                        r = mybir.dt.float32r
                        nc.tensor.matmul(pslice,
                                         x1_sb[:, h * W:(h + 1) * W].bitcast(r),
                                         x2_sb[:, r0:r1].bitcast(r),
                                         start=True, stop=True)
                    else:
                        nc.tensor.matmul(pslice, x1_sb[:, h * W:(h + 1) * W],
                                         x2_sb[:, r0:r1],
                                         start=True, stop=True)
                    dst = stage[:, dy0 * W:(dy0 + nb) * W]
                    # drain PSUM on both DVE and ACT so neither is the
                    # bottleneck (DVE ~2x faster than ACT at f32 copies)
                    if b % 3 == 2:
                        nc.scalar.copy(dst, pslice)
                    else:
                        nc.vector.tensor_copy(dst, pslice)
                nc.sync.dma_start(corr[h], stage[:])

    nc.compile()
    return nc


_NC_CACHE = None


def _get_nc():
    global _NC_CACHE
    if _NC_CACHE is None:
        _NC_CACHE = _build_nc()
    return _NC_CACHE


def _prep_in_maps(x_1, x_2, img, p_1, p_2):
    x1cat = np.concatenate([x_1[0], p_1[0]], axis=0).astype(np.float32)  # [128,96,96]
    x2cat = np.concatenate([x_2[0], p_2[0]], axis=0).astype(np.float32)
    x2pad = np.zeros((C2, H + 2 * PAD, W), np.float32)
    x2pad[:, PAD:PAD + H] = x2cat
    imgp = np.zeros((H + 2 * (PAD + 1), W), np.float32)
    imgp[PAD + 1:PAD + 1 + H] = img[0, 0]

    if MM_DTYPE == "bf16x3":
        import ml_dtypes
        bf = ml_dtypes.bfloat16

        def split(a):
            hi = a.astype(bf)
            lo = (a - hi.astype(np.float32)).astype(bf)
            return hi, lo

        x1h, x1l = split(x1cat)
        x2h, x2l = split(x2pad)

    in_maps = []
    for i in range(NCORES):
        h0 = i * HS
        m = {"img": np.ascontiguousarray(imgp[h0:h0 + IMR]).reshape(-1)}
        if MM_DTYPE == "bf16x3":
            m["x1h"] = np.ascontiguousarray(x1h[:, h0:h0 + HS]).reshape(C2, HS * W)
            m["x1l"] = np.ascontiguousarray(x1l[:, h0:h0 + HS]).reshape(C2, HS * W)
            m["x2h"] = np.ascontiguousarray(x2h[:, h0:h0 + X2R]).reshape(C2, X2R * W)
            m["x2l"] = np.ascontiguousarray(x2l[:, h0:h0 + X2R]).reshape(C2, X2R * W)
        else:
            m["x1"] = np.ascontiguousarray(x1cat[:, h0:h0 + HS]).reshape(C2, HS * W)
            m["x2"] = np.ascontiguousarray(x2pad[:, h0:h0 + X2R]).reshape(C2, X2R * W)
        in_maps.append(m)
    return in_maps


_DXW = np.add.outer(np.arange(D), np.arange(W))  # dx + w
_WMASK = ((_DXW >= PAD) & (_DXW < PAD + W)).astype(np.float32)[None, :, None, :]


def _postprocess(results):
    vb_parts, img_parts = [], []
    for i in range(NCORES):
        corr = np.asarray(results[i]["corr"])  # [HS, W, D, W] = [h, w, dy, wp]
        buf = np.zeros((HS, W, D, W + 2 * PAD), np.float32)
        buf[:, :, :, PAD:PAD + W] = corr
        s = buf.strides
        # v[dy, dx, h, w] = buf[h, w, dy, w + dx]; w+dx out of [PAD, PAD+W)
        # lands in the zero margins -> the band clip mask comes for free.
        v = np.lib.stride_tricks.as_strided(
            buf, shape=(D, D, HS, W),
            strides=(s[2], s[3], s[0], s[1] + s[3]))
        vb_parts.append(np.ascontiguousarray(v).reshape(D * D, HS, W))

        im = np.asarray(results[i]["imgo"]).reshape(D, D, HS, W) * _WMASK
        img_parts.append(im.reshape(D * D, HS, W))

    out_vb = np.concatenate(vb_parts, axis=1)[None]
    out_img = np.concatenate(img_parts, axis=1)[None]
    return out_vb, out_img


def kernel(x_1, x_2, img, p_1, p_2, _trace=False):
    nc = _get_nc()
    in_maps = _prep_in_maps(np.asarray(x_1), np.asarray(x_2), np.asarray(img),
                            np.asarray(p_1), np.asarray(p_2))
    res = run_bass_kernel_spmd(nc, in_maps, list(range(NCORES)), trace=_trace)
    out = _postprocess(res.results)
    if _trace:
        return out, res
    return out
